# revision 18
# baseline (speedup 1.0000x reference)
"""HDClassifier Trainium2 kernel (v2).

Math (per batch b):
  idx[t,c]   = clip(round((x+100)/200*200), 0, 200)
  bundled[t] = sum_c level_hv[idx[t,c]] * channel_hv[c]          # ints in [-8,8]
  gram[t',d] = prod_{i=0..3} bundled[t'+i, (d-(3-i)) mod D]      # |.| <= 4096
  sample[d]  = sum_{t'=0..124} gram[t',d]
  out        = sign(sample) @ centroid.T

Device strategy (8 cores, 4 batches each):
  - Host compacts the folded table per core: only the ~1350 (channel,level)
    keys actually used by the core's 4 batches are uploaded, padded to
    KP*256 rows (KP=6 expected) -> 6 DoubleRow k-passes instead of 7.
  - Rows carry a 3-col circular halo on the left (cols 9997..9999,0..9999),
    so the n-gram's circular d-shifts become plain free-dim offsets.
  - Phase A (chunk-major, table streamed): per 512-col chunk, per batch,
    KP fp8 DoubleRow matmuls accumulate onehot.T @ table in PSUM; Act
    drains to a per-batch fp8 bundled tile [128, 10003].
  - Phase B (per quarter, per batch): DMA-stage the partition-shifted
    bundled (sh1) and u (ush); DVE: u = bund*sh1 (fp16), gram = u*ush
    (fp16, values <= 4096 exact-ish in fp16).
  - t'-reduce: one fp16 matmul per (batch, chunk) with a ones-column
    lhsT view selecting row 20b+c of a single [80,512] f32 PSUM bank
    accumulated across all 80 matmuls -> one drain + one output DMA.
  - Host: sign + tiny [32,10000]@[10000,6] matmul.
"""

import sys

sys.path.insert(0, "/opt/trn_rl_repo")

import numpy as np

import concourse.bass as bass
import concourse.mybir as mybir
from concourse import bacc
from concourse.bass_utils import run_bass_kernel_spmd
from concourse.tile import TileContext

# Problem constants (hardcoded per contract)
NUM_LEVELS = 201
N_GRAM = 4
B, T, C, D, NUM_CLASSES = 32, 128, 8, 10000, 6
N_CORES = 8
B_LOC = B // N_CORES  # 4 batches per core
K_TOT = C * NUM_LEVELS  # 1608
HALO = N_GRAM - 1  # 3
DL = D + HALO  # 10003 local bundled width

CH = 512
NCH = 20  # out-chunk grid: 19x512 + 272 over D
CHW_B = [min(CH, D - CH * c) for c in range(NCH)]  # out-chunk widths (last 272)
# phase-A chunk grid over DL=10003: 19x512, a 3-col sliver, then 272.
# The sliver lets the 4th phase-B group (out-chunks 15-18, needing bundled
# cols <= 9730) start before the last 272-col chunk, so only the final
# 272-col group is exposed as tail.
A_STARTS = [0, 128] + [CH * c for c in range(1, 19)] + [9728, 9731]
A_WIDTHS = [128, CH - 128] + [CH] * 18 + [3, DL - 9731]
NCA = len(A_STARTS)  # 21
# phase-B slices: (first out-chunk, n chunks, ready after A-chunk index).
# A slice ending at out-chunk e reads bundled cols <= 512(e+1)+2, i.e.
# A-chunk e+1 (the sliver, index 19, for e=18; the last A-chunk for e=19).
# 3-chunk slices start phase B at A-chunk 3 and leave only a 272-col tail.
QDEF = [
    (0, 3, 4),
    (3, 3, 7),
    (6, 3, 10),
    (9, 3, 13),
    (12, 3, 16),
    (15, 3, 19),
    (18, 1, 20),
    (19, 1, 21),
]
QMAX = 3 * CH + 1  # largest slice p/sh2 width (+1 overhang)

FP8 = mybir.dt.float8e4
FP16 = mybir.dt.float16
F32 = mybir.dt.float32
NP_FP8 = np.dtype(mybir.dt.np(FP8))
NP_FP16 = np.dtype(mybir.dt.np(FP16))

_CACHE = {}


def _build_program(kp):
    nc = bacc.Bacc("TRN2", target_bir_lowering=False, debug=False, num_devices=N_CORES)

    table_p = nc.declare_dram_parameter("table", [128, kp, 2, DL], FP8, isOutput=False)
    oh_p = nc.declare_dram_parameter("onehot", [128, B_LOC, kp, 2, T], FP8, isOutput=False)
    eb_p = nc.declare_dram_parameter("eb", [128, 2 * 80 + 1], FP16, isOutput=False)
    out_p = nc.declare_dram_parameter("sample", [80, CH], F32, isOutput=True)

    with TileContext(nc) as tc:
        with (
            tc.tile_pool(name="const", bufs=1) as cpool,
            tc.tile_pool(name="tab", bufs=4) as tpool,
            tc.tile_pool(name="bund", bufs=1) as bpool,
            tc.tile_pool(name="sh2", bufs=2) as shpool,
            tc.tile_pool(name="pp", bufs=2) as ppool,
            tc.tile_pool(name="psh", bufs=2) as pshpool,
            tc.tile_pool(name="gram", bufs=4) as gpool,
            tc.tile_pool(name="psA", bufs=6, space="PSUM") as psA_pool,
            tc.tile_pool(name="psB", bufs=1, space="PSUM") as psB_pool,
        ):
            oh_sb = cpool.tile([128, B_LOC, kp, 2, T], FP8, tag="oh")
            nc.sync.dma_start(out=oh_sb[:, 0], in_=oh_p[:, 0])
            eb_sb = cpool.tile([128, 2 * 80 + 1], FP16, tag="eb")

            # all batches' bundled side by side: [t, b, col]
            bund = bpool.tile([128, B_LOC, DL], FP8, tag="bund", name="bund")
            psBIG = psB_pool.tile([80, CH], F32, tag="psBIG")
            nred = [0]  # count of reduce matmuls emitted

            deferred = []  # (gram tile, c) whose reduce matmuls wait a slice

            def emit_reduce(gram, c):
                w = CHW_B[c]
                for b in range(B_LOC):
                    r = b * NCH + c
                    nc.tensor.matmul(
                        psBIG[:, 0:w],
                        eb_sb[:125, 80 - r : 160 - r],
                        gram[:125, b, 0:w],
                        start=(nred[0] == 0),
                        stop=(nred[0] == B_LOC * NCH - 1),
                    )
                    nred[0] += 1

            def flush_deferred():
                while deferred:
                    emit_reduce(*deferred.pop(0))

            def phase_b_slice(q):
                """gram[t',d] = p[t',d] * p[t'+1,d+1] with
                p[t,m] = bund[t,m]*bund[t+2,m+2]; p is exact in fp8
                (products of even ints <= 64), so both staged shifts are
                fp8 and all 4 batches share one DMA / one DVE op. The
                middle chunk's gram runs on GPSIMD (Pool) with its reduce
                matmuls deferred one slice so the in-order PE queue never
                waits on Pool."""
                c_first, n_ch, _ = QDEF[q]
                q0 = CH * c_first  # d-offset of slice
                wq = CH * (n_ch - 1) + CHW_B[c_first + n_ch - 1]
                # sh2[t, b, j] = bund[t+2, b, q0+j+2], j in [0, wq+1)
                sh2 = shpool.tile([128, B_LOC, QMAX], FP8, tag="sh2")
                nc.sync.dma_start(
                    out=sh2[:126, :, 0 : wq + 1],
                    in_=bund[2:128, :, q0 + 2 : q0 + wq + 3],
                )
                # p[t, b, m] = bund[t, b, q0+m] * bund[t+2, b, q0+m+2]
                p = ppool.tile([128, B_LOC, QMAX], FP8, tag="p")
                nc.vector.tensor_mul(
                    out=p[:126, :, 0 : wq + 1],
                    in0=bund[:126, :, q0 : q0 + wq + 1],
                    in1=sh2[:126, :, 0 : wq + 1],
                )
                # psh[t, b, j] = p[t+1, b, j+1]
                psh = pshpool.tile([128, B_LOC, QMAX - 1], FP8, tag="psh")
                nc.sync.dma_start(
                    out=psh[:125, :, 0:wq], in_=p[1:126, :, 1 : wq + 1]
                )
                flush_deferred()
                for l in range(n_ch):
                    c = c_first + l
                    w = CHW_B[c]
                    off = CH * l
                    gram = gpool.tile([128, B_LOC, CH], FP16, tag="gram")
                    on_pool = n_ch == 3 and l == 1
                    eng = nc.gpsimd if on_pool else nc.vector
                    eng.tensor_mul(
                        out=gram[:125, :, 0:w],
                        in0=p[:125, :, off : off + w],
                        in1=psh[:125, :, off : off + w],
                    )
                    if on_pool:
                        deferred.append((gram, c))
                    else:
                        emit_reduce(gram, c)

            schedule = {}
            for q, (_, _, ready) in enumerate(QDEF):
                schedule.setdefault(min(ready, NCA - 1), []).append(q)

            for ac in range(NCA):
                w = A_WIDTHS[ac]
                c0 = A_STARTS[ac]
                tab = tpool.tile([128, kp, 2, CH], FP8, tag="tab")
                nc.sync.dma_start(
                    out=tab[:, :, :, 0:w], in_=table_p[:, :, :, c0 : c0 + w]
                )
                if ac == 0:
                    for b in range(1, B_LOC):
                        nc.sync.dma_start(out=oh_sb[:, b], in_=oh_p[:, b])
                if ac == 1:
                    nc.sync.dma_start(out=eb_sb[:], in_=eb_p[:])
                for b in range(B_LOC):
                    ps = psA_pool.tile([128, w], F32, tag="psA", name=f"psA{ac}_{b}")
                    for k in range(kp):
                        nc.tensor.matmul(
                            ps[:],
                            oh_sb[:, b, k, :, :],
                            tab[:, k, :, 0:w],
                            start=(k == 0),
                            stop=(k == kp - 1),
                            perf_mode=mybir.MatmulPerfMode.DoubleRow,
                        )
                    nc.scalar.copy(out=bund[:, b, c0 : c0 + w], in_=ps[:])
                for q in schedule.get(ac, []):
                    phase_b_slice(q)

            flush_deferred()
            samp = cpool.tile([80, CH], F32, tag="samp")
            nc.scalar.copy(out=samp[:], in_=psBIG[:])
            nc.sync.dma_start(out=out_p[:], in_=samp[:])

    nc.finalize()
    return nc


def _host_prep(x, level_hv, channel_hv):
    # Bit-exact replication of the jax fp32 quantization
    x = np.asarray(x, dtype=np.float32)
    t1 = x + np.float32(100.0)
    t2 = t1 / np.float32(200.0)
    t3 = t2 * np.float32(200.0)
    idx = np.clip(np.rint(t3), 0, NUM_LEVELS - 1).astype(np.int32)  # [B,T,C]

    fp8_one = np.array([1.0], dtype=np.float32).astype(NP_FP8)[0]
    fp8_mone = np.array([-1.0], dtype=np.float32).astype(NP_FP8)[0]

    # folded +-1 table as fp8 bytes [1608, D]
    prod = (level_hv[None, :, :] * channel_hv[:, None, :]).reshape(K_TOT, D)
    F = np.where(prod > 0, fp8_one, fp8_mone)

    kk = np.arange(C, dtype=np.int32)[None, None, :] * NUM_LEVELS + idx  # [B,T,C]

    cores = []
    kp_max = 1
    for core in range(N_CORES):
        kk_c = kk[core * B_LOC : (core + 1) * B_LOC]  # [B_LOC, T, C]
        keys = np.unique(kk_c)
        n_k = len(keys)
        kp_c = -(-n_k // 256)
        kp_max = max(kp_max, kp_c)
        cores.append((kk_c, keys, n_k))

    kp = kp_max
    kpad = kp * 256
    in_maps = []
    eb = np.zeros((128, 2 * 80 + 1), dtype=NP_FP16)
    eb[: T - N_GRAM + 1, 80] = np.float16(1.0)
    for kk_c, keys, n_k in cores:
        inv = np.zeros(K_TOT, dtype=np.int32)
        inv[keys] = np.arange(n_k, dtype=np.int32)
        slots = inv[kk_c]  # [B_LOC, T, C]

        tabc = np.zeros((kpad, DL), dtype=NP_FP8)
        tabc[:n_k, HALO:] = F[keys]
        tabc[:n_k, :HALO] = F[keys][:, D - HALO :]
        table_up = np.ascontiguousarray(
            tabc.reshape(kp, 2, 128, DL).transpose(2, 0, 1, 3)
        )  # [128, kp, 2, DL]

        oh = np.zeros((B_LOC, kpad, T), dtype=NP_FP8)
        bb, tt, cc = np.meshgrid(
            np.arange(B_LOC), np.arange(T), np.arange(C), indexing="ij"
        )
        oh[bb.ravel(), slots.ravel(), tt.ravel()] = fp8_one
        oh_up = np.ascontiguousarray(
            oh.reshape(B_LOC, kp, 2, 128, T).transpose(3, 0, 1, 2, 4)
        )  # [128, B_LOC, kp, 2, T]

        in_maps.append({"table": table_up, "onehot": oh_up, "eb": eb})
    return kp, in_maps


def kernel(x, level_hv, channel_hv, centroid):
    kp, in_maps = _host_prep(x, level_hv, channel_hv)
    if kp not in _CACHE:
        _CACHE[kp] = _build_program(kp)
    nc = _CACHE[kp]

    res = run_bass_kernel_spmd(nc, in_maps, list(range(N_CORES)))
    _CACHE["last_results"] = res
    _CACHE["nc"] = nc

    sample = np.empty((B, D), dtype=np.float32)
    for core in range(N_CORES):
        arr = res.results[core]["sample"]  # [80, 512]
        for b in range(B_LOC):
            row = arr[b * NCH : (b + 1) * NCH]  # [20, 512]
            for c in range(NCH):
                w = CHW_B[c]
                sample[core * B_LOC + b, CH * c : CH * c + w] = row[c, :w]
    sign = np.where(sample > 0, np.float32(1.0), np.float32(-1.0))
    return (sign @ np.asarray(centroid, dtype=np.float32).T).astype(np.float32)


# revision 24
# speedup vs baseline: 1.0680x; 1.0680x over previous
"""HDClassifier Trainium2 kernel (v2).

Math (per batch b):
  idx[t,c]   = clip(round((x+100)/200*200), 0, 200)
  bundled[t] = sum_c level_hv[idx[t,c]] * channel_hv[c]          # ints in [-8,8]
  gram[t',d] = prod_{i=0..3} bundled[t'+i, (d-(3-i)) mod D]      # |.| <= 4096
  sample[d]  = sum_{t'=0..124} gram[t',d]
  out        = sign(sample) @ centroid.T

Device strategy (8 cores, 4 batches each):
  - Host compacts the folded table per core: only the ~1350 (channel,level)
    keys actually used by the core's 4 batches are uploaded, padded to
    KP*256 rows (KP=6 expected) -> 6 DoubleRow k-passes instead of 7.
  - Rows carry a 3-col circular halo on the left (cols 9997..9999,0..9999),
    so the n-gram's circular d-shifts become plain free-dim offsets.
  - Phase A (chunk-major, table streamed): per 512-col chunk, per batch,
    KP fp8 DoubleRow matmuls accumulate onehot.T @ table in PSUM; Act
    drains to a per-batch fp8 bundled tile [128, 10003].
  - Phase B (per quarter, per batch): DMA-stage the partition-shifted
    bundled (sh1) and u (ush); DVE: u = bund*sh1 (fp16), gram = u*ush
    (fp16, values <= 4096 exact-ish in fp16).
  - t'-reduce: one fp16 matmul per (batch, chunk) with a ones-column
    lhsT view selecting row 20b+c of a single [80,512] f32 PSUM bank
    accumulated across all 80 matmuls -> one drain + one output DMA.
  - Host: sign + tiny [32,10000]@[10000,6] matmul.
"""

import sys

sys.path.insert(0, "/opt/trn_rl_repo")

import numpy as np

import concourse.bass as bass
import concourse.mybir as mybir
from concourse import bacc
from concourse.bass_utils import run_bass_kernel_spmd
from concourse.tile import TileContext

# Problem constants (hardcoded per contract)
NUM_LEVELS = 201
N_GRAM = 4
B, T, C, D, NUM_CLASSES = 32, 128, 8, 10000, 6
N_CORES = 8
B_LOC = B // N_CORES  # 4 batches per core
K_TOT = C * NUM_LEVELS  # 1608
HALO = N_GRAM - 1  # 3
DL = D + HALO  # 10003 local bundled width

CH = 512
NCH = 20  # out-chunk grid: 19x512 + 272 over D
CHW_B = [min(CH, D - CH * c) for c in range(NCH)]  # out-chunk widths (last 272)
# phase-A chunk grid over DL=10003: 19x512, a 3-col sliver, then 272.
# The sliver lets the 4th phase-B group (out-chunks 15-18, needing bundled
# cols <= 9730) start before the last 272-col chunk, so only the final
# 272-col group is exposed as tail.
A_STARTS = [CH * c for c in range(19)] + [9728, 9731]
A_WIDTHS = [CH] * 19 + [3, DL - 9731]
NCA = len(A_STARTS)  # 21
# phase-B slices: (first out-chunk, n chunks, ready after A-chunk index).
# A slice ending at out-chunk e reads bundled cols <= 512(e+1)+2, i.e.
# A-chunk e+1 (the sliver, index 19, for e=18; the last A-chunk for e=19).
# 3-chunk slices start phase B at A-chunk 3 and leave only a 272-col tail.
QDEF = [
    (0, 3, 3),
    (3, 3, 6),
    (6, 3, 9),
    (9, 3, 12),
    (12, 3, 15),
    (15, 3, 18),
    (18, 1, 19),
    (19, 1, 20),
]


FP8 = mybir.dt.float8e4
FP16 = mybir.dt.float16
F32 = mybir.dt.float32
NP_FP8 = np.dtype(mybir.dt.np(FP8))
NP_FP16 = np.dtype(mybir.dt.np(FP16))

# tuning knobs (swept externally)
TAB_BUFS = 4
PSA_BUFS = 6
GRAM_BUFS = 4
STAGE_BUFS = 3
POOL_LS = (1,)  # which chunk indices of 3-chunk slices run their gram on Pool
FINE_FROM = 15  # out-chunks >= this are processed per-batch (endgame)

_CACHE = {}


def _build_program(kp):
    nc = bacc.Bacc("TRN2", target_bir_lowering=False, debug=False, num_devices=N_CORES)

    table_p = nc.declare_dram_parameter("table", [128, kp, 2, DL], FP8, isOutput=False)
    oh_p = nc.declare_dram_parameter("onehot", [128, B_LOC, kp, 2, T], FP8, isOutput=False)
    eb_p = nc.declare_dram_parameter("eb", [128, 2 * 80 + 1], FP16, isOutput=False)
    out_p = nc.declare_dram_parameter("sample", [80, CH], F32, isOutput=True)

    qmax = max(CH * (n - 1) + CHW_B[f + n - 1] for f, n, _ in QDEF) + 1
    with TileContext(nc) as tc:
        with (
            tc.tile_pool(name="const", bufs=1) as cpool,
            tc.tile_pool(name="tab", bufs=TAB_BUFS) as tpool,
            tc.tile_pool(name="bund", bufs=1) as bpool,
            tc.tile_pool(name="sh2", bufs=STAGE_BUFS) as shpool,
            tc.tile_pool(name="pp", bufs=STAGE_BUFS) as ppool,
            tc.tile_pool(name="psh", bufs=STAGE_BUFS) as pshpool,
            tc.tile_pool(name="gram", bufs=GRAM_BUFS) as gpool,
            tc.tile_pool(name="psA", bufs=PSA_BUFS, space="PSUM") as psA_pool,
            tc.tile_pool(name="psB", bufs=1, space="PSUM") as psB_pool,
        ):
            oh_sb = cpool.tile([128, B_LOC, kp, 2, T], FP8, tag="oh")
            nc.sync.dma_start(out=oh_sb[:, 0], in_=oh_p[:, 0])
            eb_sb = cpool.tile([128, 2 * 80 + 1], FP16, tag="eb")

            # all batches' bundled side by side: [t, b, col]
            bund = bpool.tile([128, B_LOC, DL], FP8, tag="bund", name="bund")
            psBIG = psB_pool.tile([80, CH], F32, tag="psBIG")
            nred = [0]  # count of reduce matmuls emitted

            deferred = []  # (gram tile, c, b) whose reduce matmul waits a slice

            def emit_reduce(gram, c, b):
                w = CHW_B[c]
                r = b * NCH + c
                nc.tensor.matmul(
                    psBIG[:, 0:w],
                    eb_sb[:125, 80 - r : 160 - r],
                    gram[:125, b, 0:w],
                    start=(nred[0] == 0),
                    stop=(nred[0] == B_LOC * NCH - 1),
                )
                nred[0] += 1

            def flush_deferred():
                while deferred:
                    emit_reduce(*deferred.pop(0))

            def phase_b_slice(q):
                """gram[t',d] = p[t',d] * p[t'+1,d+1] with
                p[t,m] = bund[t,m]*bund[t+2,m+2]; p is exact in fp8
                (products of even ints <= 64), so the staged shifts are fp8.
                ALL reduce matmuls are deferred one slice so the in-order PE
                queue never waits on a gram. Endgame slices (FINE_FROM) are
                processed per batch with alternating DVE/Pool grams to keep
                the dependency chains short where no phase-A work remains."""
                c_first, n_ch, _ = QDEF[q]
                q0 = CH * c_first  # d-offset of slice
                wq = CH * (n_ch - 1) + CHW_B[c_first + n_ch - 1]
                fine = c_first >= FINE_FROM
                # sh2[t, b, j] = bund[t+2, b, q0+j+2], j in [0, wq+1)
                sh2 = shpool.tile([128, B_LOC, qmax], FP8, tag="sh2")
                nc.sync.dma_start(
                    out=sh2[:126, :, 0 : wq + 1],
                    in_=bund[2:128, :, q0 + 2 : q0 + wq + 3],
                )
                p = ppool.tile([128, B_LOC, qmax], FP8, tag="p")
                psh = pshpool.tile([128, B_LOC, qmax - 1], FP8, tag="psh")
                bsplit = [range(B_LOC)] if not fine else [[b] for b in range(B_LOC)]
                first = True
                for bs in bsplit:
                    blo, bhi = bs[0], bs[-1] + 1
                    # p[t, b, m] = bund[t, b, q0+m] * bund[t+2, b, q0+m+2]
                    nc.vector.tensor_mul(
                        out=p[:126, blo:bhi, 0 : wq + 1],
                        in0=bund[:126, blo:bhi, q0 : q0 + wq + 1],
                        in1=sh2[:126, blo:bhi, 0 : wq + 1],
                    )
                    # psh[t, b, j] = p[t+1, b, j+1]
                    nc.sync.dma_start(
                        out=psh[:125, blo:bhi, 0:wq],
                        in_=p[1:126, blo:bhi, 1 : wq + 1],
                    )
                    if first:
                        flush_deferred()
                        first = False
                    for l in range(n_ch):
                        c = c_first + l
                        w = CHW_B[c]
                        off = CH * l
                        gram = gpool.tile([128, B_LOC, CH], FP16, tag="gram")
                        if fine:
                            on_pool = blo % 2 == 1
                        else:
                            on_pool = n_ch == 3 and l in POOL_LS
                        eng = nc.gpsimd if on_pool else nc.vector
                        eng.tensor_mul(
                            out=gram[:125, blo:bhi, 0:w],
                            in0=p[:125, blo:bhi, off : off + w],
                            in1=psh[:125, blo:bhi, off : off + w],
                        )
                        for b in bs:
                            deferred.append((gram, c, b))

            schedule = {}
            for q, (_, _, ready) in enumerate(QDEF):
                schedule.setdefault(min(ready, NCA - 1), []).append(q)

            for ac in range(NCA):
                w = A_WIDTHS[ac]
                c0 = A_STARTS[ac]
                tab = tpool.tile([128, kp, 2, CH], FP8, tag="tab")
                nc.sync.dma_start(
                    out=tab[:, :, :, 0:w], in_=table_p[:, :, :, c0 : c0 + w]
                )
                if ac == 0:
                    for b in range(1, B_LOC):
                        nc.sync.dma_start(out=oh_sb[:, b], in_=oh_p[:, b])
                if ac == 1:
                    nc.sync.dma_start(out=eb_sb[:], in_=eb_p[:])
                for b in range(B_LOC):
                    ps = psA_pool.tile([128, w], F32, tag="psA", name=f"psA{ac}_{b}")
                    for k in range(kp):
                        nc.tensor.matmul(
                            ps[:],
                            oh_sb[:, b, k, :, :],
                            tab[:, k, :, 0:w],
                            start=(k == 0),
                            stop=(k == kp - 1),
                            perf_mode=mybir.MatmulPerfMode.DoubleRow,
                        )
                    nc.scalar.copy(out=bund[:, b, c0 : c0 + w], in_=ps[:])
                for q in schedule.get(ac, []):
                    phase_b_slice(q)

            flush_deferred()
            samp = cpool.tile([80, CH], F32, tag="samp")
            nc.scalar.copy(out=samp[:], in_=psBIG[:])
            nc.sync.dma_start(out=out_p[:], in_=samp[:])

    nc.finalize()
    return nc


def _host_prep(x, level_hv, channel_hv):
    # Bit-exact replication of the jax fp32 quantization
    x = np.asarray(x, dtype=np.float32)
    t1 = x + np.float32(100.0)
    t2 = t1 / np.float32(200.0)
    t3 = t2 * np.float32(200.0)
    idx = np.clip(np.rint(t3), 0, NUM_LEVELS - 1).astype(np.int32)  # [B,T,C]

    fp8_one = np.array([1.0], dtype=np.float32).astype(NP_FP8)[0]
    fp8_mone = np.array([-1.0], dtype=np.float32).astype(NP_FP8)[0]

    # folded +-1 table as fp8 bytes [1608, D]
    prod = (level_hv[None, :, :] * channel_hv[:, None, :]).reshape(K_TOT, D)
    F = np.where(prod > 0, fp8_one, fp8_mone)

    kk = np.arange(C, dtype=np.int32)[None, None, :] * NUM_LEVELS + idx  # [B,T,C]

    cores = []
    kp_max = 1
    for core in range(N_CORES):
        kk_c = kk[core * B_LOC : (core + 1) * B_LOC]  # [B_LOC, T, C]
        keys = np.unique(kk_c)
        n_k = len(keys)
        kp_c = -(-n_k // 256)
        kp_max = max(kp_max, kp_c)
        cores.append((kk_c, keys, n_k))

    kp = kp_max
    kpad = kp * 256
    in_maps = []
    eb = np.zeros((128, 2 * 80 + 1), dtype=NP_FP16)
    eb[: T - N_GRAM + 1, 80] = np.float16(1.0)
    for kk_c, keys, n_k in cores:
        inv = np.zeros(K_TOT, dtype=np.int32)
        inv[keys] = np.arange(n_k, dtype=np.int32)
        slots = inv[kk_c]  # [B_LOC, T, C]

        tabc = np.zeros((kpad, DL), dtype=NP_FP8)
        tabc[:n_k, HALO:] = F[keys]
        tabc[:n_k, :HALO] = F[keys][:, D - HALO :]
        table_up = np.ascontiguousarray(
            tabc.reshape(kp, 2, 128, DL).transpose(2, 0, 1, 3)
        )  # [128, kp, 2, DL]

        oh = np.zeros((B_LOC, kpad, T), dtype=NP_FP8)
        bb, tt, cc = np.meshgrid(
            np.arange(B_LOC), np.arange(T), np.arange(C), indexing="ij"
        )
        oh[bb.ravel(), slots.ravel(), tt.ravel()] = fp8_one
        oh_up = np.ascontiguousarray(
            oh.reshape(B_LOC, kp, 2, 128, T).transpose(3, 0, 1, 2, 4)
        )  # [128, B_LOC, kp, 2, T]

        in_maps.append({"table": table_up, "onehot": oh_up, "eb": eb})
    return kp, in_maps


def kernel(x, level_hv, channel_hv, centroid):
    kp, in_maps = _host_prep(x, level_hv, channel_hv)
    if kp not in _CACHE:
        _CACHE[kp] = _build_program(kp)
    nc = _CACHE[kp]

    res = run_bass_kernel_spmd(nc, in_maps, list(range(N_CORES)))
    _CACHE["last_results"] = res
    _CACHE["nc"] = nc

    sample = np.empty((B, D), dtype=np.float32)
    for core in range(N_CORES):
        arr = res.results[core]["sample"]  # [80, 512]
        for b in range(B_LOC):
            row = arr[b * NCH : (b + 1) * NCH]  # [20, 512]
            for c in range(NCH):
                w = CHW_B[c]
                sample[core * B_LOC + b, CH * c : CH * c + w] = row[c, :w]
    sign = np.where(sample > 0, np.float32(1.0), np.float32(-1.0))
    return (sign @ np.asarray(centroid, dtype=np.float32).T).astype(np.float32)


# revision 25
# speedup vs baseline: 1.0857x; 1.0166x over previous
"""HDClassifier Trainium2 kernel (v2).

Math (per batch b):
  idx[t,c]   = clip(round((x+100)/200*200), 0, 200)
  bundled[t] = sum_c level_hv[idx[t,c]] * channel_hv[c]          # ints in [-8,8]
  gram[t',d] = prod_{i=0..3} bundled[t'+i, (d-(3-i)) mod D]      # |.| <= 4096
  sample[d]  = sum_{t'=0..124} gram[t',d]
  out        = sign(sample) @ centroid.T

Device strategy (8 cores, 4 batches each):
  - Host compacts the folded table per core: only the ~1350 (channel,level)
    keys actually used by the core's 4 batches are uploaded, padded to
    KP*256 rows (KP=6 expected) -> 6 DoubleRow k-passes instead of 7.
  - Rows carry a 3-col circular halo on the left (cols 9997..9999,0..9999),
    so the n-gram's circular d-shifts become plain free-dim offsets.
  - Phase A (chunk-major, table streamed): per 512-col chunk, per batch,
    KP fp8 DoubleRow matmuls accumulate onehot.T @ table in PSUM; Act
    drains to a per-batch fp8 bundled tile [128, 10003].
  - Phase B (per quarter, per batch): DMA-stage the partition-shifted
    bundled (sh1) and u (ush); DVE: u = bund*sh1 (fp16), gram = u*ush
    (fp16, values <= 4096 exact-ish in fp16).
  - t'-reduce: one fp16 matmul per (batch, chunk) with a ones-column
    lhsT view selecting row 20b+c of a single [80,512] f32 PSUM bank
    accumulated across all 80 matmuls -> one drain + one output DMA.
  - Host: sign + tiny [32,10000]@[10000,6] matmul.
"""

import sys

sys.path.insert(0, "/opt/trn_rl_repo")

import numpy as np

import concourse.bass as bass
import concourse.mybir as mybir
from concourse import bacc
from concourse.bass_utils import run_bass_kernel_spmd
from concourse.tile import TileContext

# Problem constants (hardcoded per contract)
NUM_LEVELS = 201
N_GRAM = 4
B, T, C, D, NUM_CLASSES = 32, 128, 8, 10000, 6
N_CORES = 8
B_LOC = B // N_CORES  # 4 batches per core
K_TOT = C * NUM_LEVELS  # 1608
HALO = N_GRAM - 1  # 3
DL = D + HALO  # 10003 local bundled width

CH = 512
NCH = 20  # out-chunk grid: 19x512 + 272 over D
CHW_B = [min(CH, D - CH * c) for c in range(NCH)]  # out-chunk widths (last 272)
# phase-A chunk grid over DL=10003: 19x512, a 3-col sliver, then 272.
# The sliver lets the 4th phase-B group (out-chunks 15-18, needing bundled
# cols <= 9730) start before the last 272-col chunk, so only the final
# 272-col group is exposed as tail.
A_STARTS = [CH * c for c in range(19)] + [9728, 9731]
A_WIDTHS = [CH] * 19 + [3, DL - 9731]
NCA = len(A_STARTS)  # 21
# phase-B slices: (first out-chunk, n chunks, ready after A-chunk index).
# A slice ending at out-chunk e reads bundled cols <= 512(e+1)+2, i.e.
# A-chunk e+1 (the sliver, index 19, for e=18; the last A-chunk for e=19).
# 3-chunk slices start phase B at A-chunk 3 and leave only a 272-col tail.
QDEF = [
    (0, 2, 2),
    (2, 2, 4),
    (4, 2, 6),
    (6, 2, 8),
    (8, 2, 10),
    (10, 2, 12),
    (12, 2, 14),
    (14, 2, 16),
    (16, 2, 18),
    (18, 1, 19),
    (19, 1, 20),
]


FP8 = mybir.dt.float8e4
FP16 = mybir.dt.float16
F32 = mybir.dt.float32
NP_FP8 = np.dtype(mybir.dt.np(FP8))
NP_FP16 = np.dtype(mybir.dt.np(FP16))

# tuning knobs (swept externally)
TAB_BUFS = 4
PSA_BUFS = 6
GRAM_BUFS = 4
STAGE_BUFS = 4
POOL_LS = (1,)  # which chunk indices of 3-chunk slices run their gram on Pool
FINE_FROM = 0  # out-chunks >= this are processed per-batch (endgame)

_CACHE = {}


def _build_program(kp):
    nc = bacc.Bacc("TRN2", target_bir_lowering=False, debug=False, num_devices=N_CORES)

    table_p = nc.declare_dram_parameter("table", [128, kp, 2, DL], FP8, isOutput=False)
    oh_p = nc.declare_dram_parameter("onehot", [128, B_LOC, kp, 2, T], FP8, isOutput=False)
    eb_p = nc.declare_dram_parameter("eb", [128, 2 * 80 + 1], FP16, isOutput=False)
    out_p = nc.declare_dram_parameter("sample", [80, CH], F32, isOutput=True)

    qmax = max(CH * (n - 1) + CHW_B[f + n - 1] for f, n, _ in QDEF) + 1
    with TileContext(nc) as tc:
        with (
            tc.tile_pool(name="const", bufs=1) as cpool,
            tc.tile_pool(name="tab", bufs=TAB_BUFS) as tpool,
            tc.tile_pool(name="bund", bufs=1) as bpool,
            tc.tile_pool(name="sh2", bufs=STAGE_BUFS) as shpool,
            tc.tile_pool(name="pp", bufs=STAGE_BUFS) as ppool,
            tc.tile_pool(name="psh", bufs=STAGE_BUFS) as pshpool,
            tc.tile_pool(name="gram", bufs=GRAM_BUFS) as gpool,
            tc.tile_pool(name="psA", bufs=PSA_BUFS, space="PSUM") as psA_pool,
            tc.tile_pool(name="psB", bufs=1, space="PSUM") as psB_pool,
        ):
            oh_sb = cpool.tile([128, B_LOC, kp, 2, T], FP8, tag="oh")
            nc.sync.dma_start(out=oh_sb[:, 0], in_=oh_p[:, 0])
            eb_sb = cpool.tile([128, 2 * 80 + 1], FP16, tag="eb")

            # all batches' bundled side by side: [t, b, col]
            bund = bpool.tile([128, B_LOC, DL], FP8, tag="bund", name="bund")
            psBIG = psB_pool.tile([80, CH], F32, tag="psBIG")
            nred = [0]  # count of reduce matmuls emitted

            deferred = []  # (gram tile, c, b) whose reduce matmul waits a slice

            def emit_reduce(gram, c, b):
                w = CHW_B[c]
                r = b * NCH + c
                nc.tensor.matmul(
                    psBIG[:, 0:w],
                    eb_sb[:125, 80 - r : 160 - r],
                    gram[:125, b, 0:w],
                    start=(nred[0] == 0),
                    stop=(nred[0] == B_LOC * NCH - 1),
                )
                nred[0] += 1

            def flush_deferred():
                while deferred:
                    emit_reduce(*deferred.pop(0))

            def phase_b_slice(q):
                """gram[t',d] = p[t',d] * p[t'+1,d+1] with
                p[t,m] = bund[t,m]*bund[t+2,m+2]; p is exact in fp8
                (products of even ints <= 64), so the staged shifts are fp8.
                ALL reduce matmuls are deferred one slice so the in-order PE
                queue never waits on a gram. Endgame slices (FINE_FROM) are
                processed per batch with alternating DVE/Pool grams to keep
                the dependency chains short where no phase-A work remains."""
                c_first, n_ch, _ = QDEF[q]
                q0 = CH * c_first  # d-offset of slice
                wq = CH * (n_ch - 1) + CHW_B[c_first + n_ch - 1]
                fine = c_first >= FINE_FROM
                # sh2[t, b, j] = bund[t+2, b, q0+j+2], j in [0, wq+1)
                sh2 = shpool.tile([128, B_LOC, qmax], FP8, tag="sh2")
                nc.sync.dma_start(
                    out=sh2[:126, :, 0 : wq + 1],
                    in_=bund[2:128, :, q0 + 2 : q0 + wq + 3],
                )
                p = ppool.tile([128, B_LOC, qmax], FP8, tag="p")
                psh = pshpool.tile([128, B_LOC, qmax - 1], FP8, tag="psh")
                bsplit = [range(B_LOC)] if not fine else [[b] for b in range(B_LOC)]
                first = True
                for bs in bsplit:
                    blo, bhi = bs[0], bs[-1] + 1
                    # p[t, b, m] = bund[t, b, q0+m] * bund[t+2, b, q0+m+2]
                    nc.vector.tensor_mul(
                        out=p[:126, blo:bhi, 0 : wq + 1],
                        in0=bund[:126, blo:bhi, q0 : q0 + wq + 1],
                        in1=sh2[:126, blo:bhi, 0 : wq + 1],
                    )
                    # psh[t, b, j] = p[t+1, b, j+1]
                    nc.sync.dma_start(
                        out=psh[:125, blo:bhi, 0:wq],
                        in_=p[1:126, blo:bhi, 1 : wq + 1],
                    )
                    if first:
                        flush_deferred()
                        first = False
                    for l in range(n_ch):
                        c = c_first + l
                        w = CHW_B[c]
                        off = CH * l
                        gram = gpool.tile([128, B_LOC, CH], FP16, tag="gram")
                        if fine:
                            on_pool = blo % 2 == 1
                        else:
                            on_pool = n_ch == 3 and l in POOL_LS
                        eng = nc.gpsimd if on_pool else nc.vector
                        eng.tensor_mul(
                            out=gram[:125, blo:bhi, 0:w],
                            in0=p[:125, blo:bhi, off : off + w],
                            in1=psh[:125, blo:bhi, off : off + w],
                        )
                        for b in bs:
                            deferred.append((gram, c, b))

            schedule = {}
            for q, (_, _, ready) in enumerate(QDEF):
                schedule.setdefault(min(ready, NCA - 1), []).append(q)

            for ac in range(NCA):
                w = A_WIDTHS[ac]
                c0 = A_STARTS[ac]
                tab = tpool.tile([128, kp, 2, CH], FP8, tag="tab")
                nc.sync.dma_start(
                    out=tab[:, :, :, 0:w], in_=table_p[:, :, :, c0 : c0 + w]
                )
                if ac == 0:
                    for b in range(1, B_LOC):
                        nc.sync.dma_start(out=oh_sb[:, b], in_=oh_p[:, b])
                if ac == 1:
                    nc.sync.dma_start(out=eb_sb[:], in_=eb_p[:])
                for b in range(B_LOC):
                    ps = psA_pool.tile([128, w], F32, tag="psA", name=f"psA{ac}_{b}")
                    for k in range(kp):
                        nc.tensor.matmul(
                            ps[:],
                            oh_sb[:, b, k, :, :],
                            tab[:, k, :, 0:w],
                            start=(k == 0),
                            stop=(k == kp - 1),
                            perf_mode=mybir.MatmulPerfMode.DoubleRow,
                        )
                    nc.scalar.copy(out=bund[:, b, c0 : c0 + w], in_=ps[:])
                for q in schedule.get(ac, []):
                    phase_b_slice(q)

            flush_deferred()
            samp = cpool.tile([80, CH], F32, tag="samp")
            nc.scalar.copy(out=samp[:], in_=psBIG[:])
            nc.sync.dma_start(out=out_p[:], in_=samp[:])

    nc.finalize()
    return nc


def _host_prep(x, level_hv, channel_hv):
    # Bit-exact replication of the jax fp32 quantization
    x = np.asarray(x, dtype=np.float32)
    t1 = x + np.float32(100.0)
    t2 = t1 / np.float32(200.0)
    t3 = t2 * np.float32(200.0)
    idx = np.clip(np.rint(t3), 0, NUM_LEVELS - 1).astype(np.int32)  # [B,T,C]

    fp8_one = np.array([1.0], dtype=np.float32).astype(NP_FP8)[0]
    fp8_mone = np.array([-1.0], dtype=np.float32).astype(NP_FP8)[0]

    # folded +-1 table as fp8 bytes [1608, D]
    prod = (level_hv[None, :, :] * channel_hv[:, None, :]).reshape(K_TOT, D)
    F = np.where(prod > 0, fp8_one, fp8_mone)

    kk = np.arange(C, dtype=np.int32)[None, None, :] * NUM_LEVELS + idx  # [B,T,C]

    cores = []
    kp_max = 1
    for core in range(N_CORES):
        kk_c = kk[core * B_LOC : (core + 1) * B_LOC]  # [B_LOC, T, C]
        keys = np.unique(kk_c)
        n_k = len(keys)
        kp_c = -(-n_k // 256)
        kp_max = max(kp_max, kp_c)
        cores.append((kk_c, keys, n_k))

    kp = kp_max
    kpad = kp * 256
    in_maps = []
    eb = np.zeros((128, 2 * 80 + 1), dtype=NP_FP16)
    eb[: T - N_GRAM + 1, 80] = np.float16(1.0)
    for kk_c, keys, n_k in cores:
        inv = np.zeros(K_TOT, dtype=np.int32)
        inv[keys] = np.arange(n_k, dtype=np.int32)
        slots = inv[kk_c]  # [B_LOC, T, C]

        tabc = np.zeros((kpad, DL), dtype=NP_FP8)
        tabc[:n_k, HALO:] = F[keys]
        tabc[:n_k, :HALO] = F[keys][:, D - HALO :]
        table_up = np.ascontiguousarray(
            tabc.reshape(kp, 2, 128, DL).transpose(2, 0, 1, 3)
        )  # [128, kp, 2, DL]

        oh = np.zeros((B_LOC, kpad, T), dtype=NP_FP8)
        bb, tt, cc = np.meshgrid(
            np.arange(B_LOC), np.arange(T), np.arange(C), indexing="ij"
        )
        oh[bb.ravel(), slots.ravel(), tt.ravel()] = fp8_one
        oh_up = np.ascontiguousarray(
            oh.reshape(B_LOC, kp, 2, 128, T).transpose(3, 0, 1, 2, 4)
        )  # [128, B_LOC, kp, 2, T]

        in_maps.append({"table": table_up, "onehot": oh_up, "eb": eb})
    return kp, in_maps


def kernel(x, level_hv, channel_hv, centroid):
    kp, in_maps = _host_prep(x, level_hv, channel_hv)
    if kp not in _CACHE:
        _CACHE[kp] = _build_program(kp)
    nc = _CACHE[kp]

    res = run_bass_kernel_spmd(nc, in_maps, list(range(N_CORES)))
    _CACHE["last_results"] = res
    _CACHE["nc"] = nc

    sample = np.empty((B, D), dtype=np.float32)
    for core in range(N_CORES):
        arr = res.results[core]["sample"]  # [80, 512]
        for b in range(B_LOC):
            row = arr[b * NCH : (b + 1) * NCH]  # [20, 512]
            for c in range(NCH):
                w = CHW_B[c]
                sample[core * B_LOC + b, CH * c : CH * c + w] = row[c, :w]
    sign = np.where(sample > 0, np.float32(1.0), np.float32(-1.0))
    return (sign @ np.asarray(centroid, dtype=np.float32).T).astype(np.float32)


# revision 30
# speedup vs baseline: 1.1039x; 1.0168x over previous
"""HDClassifier Trainium2 kernel (v2).

Math (per batch b):
  idx[t,c]   = clip(round((x+100)/200*200), 0, 200)
  bundled[t] = sum_c level_hv[idx[t,c]] * channel_hv[c]          # ints in [-8,8]
  gram[t',d] = prod_{i=0..3} bundled[t'+i, (d-(3-i)) mod D]      # |.| <= 4096
  sample[d]  = sum_{t'=0..124} gram[t',d]
  out        = sign(sample) @ centroid.T

Device strategy (8 cores, 4 batches each):
  - Host compacts the folded table per core: only the ~1350 (channel,level)
    keys actually used by the core's 4 batches are uploaded, padded to
    KP*256 rows (KP=6 expected) -> 6 DoubleRow k-passes instead of 7.
  - Rows carry a 3-col circular halo on the left (cols 9997..9999,0..9999),
    so the n-gram's circular d-shifts become plain free-dim offsets.
  - Phase A (chunk-major, table streamed): per 512-col chunk, per batch,
    KP fp8 DoubleRow matmuls accumulate onehot.T @ table in PSUM; Act
    drains to a per-batch fp8 bundled tile [128, 10003].
  - Phase B (per quarter, per batch): DMA-stage the partition-shifted
    bundled (sh1) and u (ush); DVE: u = bund*sh1 (fp16), gram = u*ush
    (fp16, values <= 4096 exact-ish in fp16).
  - t'-reduce: one fp16 matmul per (batch, chunk) with a ones-column
    lhsT view selecting row 20b+c of a single [80,512] f32 PSUM bank
    accumulated across all 80 matmuls -> one drain + one output DMA.
  - Host: sign + tiny [32,10000]@[10000,6] matmul.
"""

import sys

sys.path.insert(0, "/opt/trn_rl_repo")

import numpy as np

import concourse.bass as bass
import concourse.mybir as mybir
from concourse import bacc
from concourse.bass_utils import run_bass_kernel_spmd
from concourse.tile import TileContext

# Problem constants (hardcoded per contract)
NUM_LEVELS = 201
N_GRAM = 4
B, T, C, D, NUM_CLASSES = 32, 128, 8, 10000, 6
N_CORES = 8
B_LOC = B // N_CORES  # 4 batches per core
K_TOT = C * NUM_LEVELS  # 1608
HALO = N_GRAM - 1  # 3
DL = D + HALO  # 10003 local bundled width

CH = 512
NCH = 20  # out-chunk grid: 19x512 + 272 over D
CHW_B = [min(CH, D - CH * c) for c in range(NCH)]  # out-chunk widths (last 272)
# phase-A chunk grid over DL=10003: 19x512, a 3-col sliver, then 272.
# The sliver lets the 4th phase-B group (out-chunks 15-18, needing bundled
# cols <= 9730) start before the last 272-col chunk, so only the final
# 272-col group is exposed as tail.
A_STARTS = [CH * c for c in range(19)] + [9728, 9731]
A_WIDTHS = [CH] * 19 + [3, DL - 9731]
NCA = len(A_STARTS)  # 21
# phase-B slices: (first out-chunk, n chunks, ready after A-chunk index).
# A slice ending at out-chunk e reads bundled cols <= 512(e+1)+2, i.e.
# A-chunk e+1 (the sliver, index 19, for e=18; the last A-chunk for e=19).
# 3-chunk slices start phase B at A-chunk 3 and leave only a 272-col tail.
QDEF = [
    (0, 2, 2),
    (2, 2, 4),
    (4, 2, 6),
    (6, 2, 8),
    (8, 2, 10),
    (10, 2, 12),
    (12, 2, 14),
    (14, 2, 16),
    (16, 2, 18),
    (18, 1, 19),
    (19, 1, 20),
]


FP8 = mybir.dt.float8e4
FP16 = mybir.dt.float16
F32 = mybir.dt.float32
NP_FP8 = np.dtype(mybir.dt.np(FP8))
NP_FP16 = np.dtype(mybir.dt.np(FP16))

# tuning knobs (swept externally)
TAB_BUFS = 4
PSA_BUFS = 6
GRAM_BUFS = 4
STAGE_BUFS = 4
POOL_LS = (1,)  # which chunk indices of 3-chunk slices run their gram on Pool
FINE_FROM = 0  # out-chunks >= this are processed per-batch (endgame)
FINE_PAIR = True  # fine slices use batch-pairs instead of singles
DIRECT_OUT = False  # DMA psBIG (PSUM) straight to DRAM, skipping the drain

_CACHE = {}


def _build_program(kp):
    nc = bacc.Bacc("TRN2", target_bir_lowering=False, debug=False, num_devices=N_CORES)

    table_p = nc.declare_dram_parameter("table", [128, kp * 2, DL], FP8, isOutput=False)
    oh_p = nc.declare_dram_parameter("onehot", [128, B_LOC, kp, 2, T], FP8, isOutput=False)
    eb_p = nc.declare_dram_parameter("eb", [128, 2 * 80 + 1], FP16, isOutput=False)
    out_p = nc.declare_dram_parameter("sample", [80, CH], F32, isOutput=True)

    qmax = max(CH * (n - 1) + CHW_B[f + n - 1] for f, n, _ in QDEF) + 1
    with TileContext(nc) as tc:
        with (
            tc.tile_pool(name="const", bufs=1) as cpool,
            tc.tile_pool(name="tab", bufs=TAB_BUFS) as tpool,
            tc.tile_pool(name="bund", bufs=1) as bpool,
            tc.tile_pool(name="sh2", bufs=STAGE_BUFS) as shpool,
            tc.tile_pool(name="pp", bufs=STAGE_BUFS) as ppool,
            tc.tile_pool(name="psh", bufs=STAGE_BUFS) as pshpool,
            tc.tile_pool(name="gram", bufs=GRAM_BUFS) as gpool,
            tc.tile_pool(name="psA", bufs=PSA_BUFS, space="PSUM") as psA_pool,
            tc.tile_pool(name="psB", bufs=1, space="PSUM") as psB_pool,
        ):
            oh_sb = cpool.tile([128, B_LOC, kp, 2, T], FP8, tag="oh")
            nc.sync.dma_start(out=oh_sb[:, 0], in_=oh_p[:, 0])
            eb_sb = cpool.tile([128, 2 * 80 + 1], FP16, tag="eb")

            # all batches' bundled side by side: [t, b, col]
            bund = bpool.tile([128, B_LOC, DL], FP8, tag="bund", name="bund")
            psBIG = psB_pool.tile([80, CH], F32, tag="psBIG")
            nred = [0]  # count of reduce matmuls emitted

            deferred = []  # (gram tile, c, b) whose reduce matmul waits a slice

            def emit_reduce(gram, c, b):
                w = CHW_B[c]
                r = b * NCH + c
                nc.tensor.matmul(
                    psBIG[:, 0:w],
                    eb_sb[:125, 80 - r : 160 - r],
                    gram[:125, b, 0:w],
                    start=(nred[0] == 0),
                    stop=(nred[0] == B_LOC * NCH - 1),
                )
                nred[0] += 1

            def flush_deferred():
                while deferred:
                    emit_reduce(*deferred.pop(0))

            def phase_b_slice(q):
                """gram[t',d] = p[t',d] * p[t'+1,d+1] with
                p[t,m] = bund[t,m]*bund[t+2,m+2]; p is exact in fp8
                (products of even ints <= 64), so the staged shifts are fp8.
                ALL reduce matmuls are deferred one slice so the in-order PE
                queue never waits on a gram. Endgame slices (FINE_FROM) are
                processed per batch with alternating DVE/Pool grams to keep
                the dependency chains short where no phase-A work remains."""
                c_first, n_ch, _ = QDEF[q]
                q0 = CH * c_first  # d-offset of slice
                wq = CH * (n_ch - 1) + CHW_B[c_first + n_ch - 1]
                fine = c_first >= FINE_FROM
                # sh2[t, b, j] = bund[t+2, b, q0+j+2], j in [0, wq+1)
                sh2 = shpool.tile([128, B_LOC, qmax], FP8, tag="sh2")
                nc.sync.dma_start(
                    out=sh2[:126, :, 0 : wq + 1],
                    in_=bund[2:128, :, q0 + 2 : q0 + wq + 3],
                )
                p = ppool.tile([128, B_LOC, qmax], FP8, tag="p")
                psh = pshpool.tile([128, B_LOC, qmax - 1], FP8, tag="psh")
                if not fine:
                    bsplit = [range(B_LOC)]
                elif FINE_PAIR:
                    bsplit = [[0, 1], [2, 3]]
                else:
                    bsplit = [[b] for b in range(B_LOC)]
                first = True
                for bs in bsplit:
                    blo, bhi = bs[0], bs[-1] + 1
                    # p[t, b, m] = bund[t, b, q0+m] * bund[t+2, b, q0+m+2]
                    nc.vector.tensor_mul(
                        out=p[:126, blo:bhi, 0 : wq + 1],
                        in0=bund[:126, blo:bhi, q0 : q0 + wq + 1],
                        in1=sh2[:126, blo:bhi, 0 : wq + 1],
                    )
                    # psh[t, b, j] = p[t+1, b, j+1]
                    nc.sync.dma_start(
                        out=psh[:125, blo:bhi, 0:wq],
                        in_=p[1:126, blo:bhi, 1 : wq + 1],
                    )
                    if first:
                        flush_deferred()
                        first = False
                    for l in range(n_ch):
                        c = c_first + l
                        w = CHW_B[c]
                        off = CH * l
                        gram = gpool.tile([128, B_LOC, CH], FP16, tag="gram")
                        if fine:
                            on_pool = (
                                (blo // 2 + l) % 2 == 1
                                if FINE_PAIR
                                else blo % 2 == 1
                            )
                        else:
                            on_pool = n_ch == 3 and l in POOL_LS
                        eng = nc.gpsimd if on_pool else nc.vector
                        eng.tensor_mul(
                            out=gram[:125, blo:bhi, 0:w],
                            in0=p[:125, blo:bhi, off : off + w],
                            in1=psh[:125, blo:bhi, off : off + w],
                        )
                        for b in bs:
                            deferred.append((gram, c, b))

            schedule = {}
            for q, (_, _, ready) in enumerate(QDEF):
                schedule.setdefault(min(ready, NCA - 1), []).append(q)

            for ac in range(NCA):
                w = A_WIDTHS[ac]
                c0 = A_STARTS[ac]
                tab = tpool.tile([128, kp * 2, CH], FP8, tag="tab")
                nc.sync.dma_start(
                    out=tab[:, :, 0:w], in_=table_p[:, :, c0 : c0 + w]
                )
                if ac == 0:
                    for b in range(1, B_LOC):
                        nc.sync.dma_start(out=oh_sb[:, b], in_=oh_p[:, b])
                if ac == 1:
                    nc.sync.dma_start(out=eb_sb[:], in_=eb_p[:])
                for b in range(B_LOC):
                    ps = psA_pool.tile([128, w], F32, tag="psA", name=f"psA{ac}_{b}")
                    for k in range(kp):
                        nc.tensor.matmul(
                            ps[:],
                            oh_sb[:, b, k, :, :],
                            tab[:, 2 * k : 2 * k + 2, 0:w],
                            start=(k == 0),
                            stop=(k == kp - 1),
                            perf_mode=mybir.MatmulPerfMode.DoubleRow,
                        )
                    nc.scalar.copy(out=bund[:, b, c0 : c0 + w], in_=ps[:])
                for q in schedule.get(ac, []):
                    phase_b_slice(q)

            flush_deferred()
            if DIRECT_OUT:
                nc.sync.dma_start(out=out_p[:], in_=psBIG[:])
                samp = None
            else:
                samp = cpool.tile([80, CH], F32, tag="samp")
                nc.scalar.copy(out=samp[:], in_=psBIG[:])
            if samp is not None:
                nc.sync.dma_start(out=out_p[:], in_=samp[:])

    nc.finalize()
    return nc


def _host_prep(x, level_hv, channel_hv):
    # Bit-exact replication of the jax fp32 quantization
    x = np.asarray(x, dtype=np.float32)
    t1 = x + np.float32(100.0)
    t2 = t1 / np.float32(200.0)
    t3 = t2 * np.float32(200.0)
    idx = np.clip(np.rint(t3), 0, NUM_LEVELS - 1).astype(np.int32)  # [B,T,C]

    fp8_one = np.array([1.0], dtype=np.float32).astype(NP_FP8)[0]
    fp8_mone = np.array([-1.0], dtype=np.float32).astype(NP_FP8)[0]

    # folded +-1 table as fp8 bytes [1608, D]
    prod = (level_hv[None, :, :] * channel_hv[:, None, :]).reshape(K_TOT, D)
    F = np.where(prod > 0, fp8_one, fp8_mone)

    kk = np.arange(C, dtype=np.int32)[None, None, :] * NUM_LEVELS + idx  # [B,T,C]

    cores = []
    kp_max = 1
    for core in range(N_CORES):
        kk_c = kk[core * B_LOC : (core + 1) * B_LOC]  # [B_LOC, T, C]
        keys = np.unique(kk_c)
        n_k = len(keys)
        kp_c = -(-n_k // 256)
        kp_max = max(kp_max, kp_c)
        cores.append((kk_c, keys, n_k))

    kp = kp_max
    kpad = kp * 256
    in_maps = []
    eb = np.zeros((128, 2 * 80 + 1), dtype=NP_FP16)
    eb[: T - N_GRAM + 1, 80] = np.float16(1.0)
    for kk_c, keys, n_k in cores:
        inv = np.zeros(K_TOT, dtype=np.int32)
        inv[keys] = np.arange(n_k, dtype=np.int32)
        slots = inv[kk_c]  # [B_LOC, T, C]

        tabc = np.zeros((kpad, DL), dtype=NP_FP8)
        tabc[:n_k, HALO:] = F[keys]
        tabc[:n_k, :HALO] = F[keys][:, D - HALO :]
        table_up = np.ascontiguousarray(
            tabc.reshape(kp, 2, 128, DL).transpose(2, 0, 1, 3)
        )  # [128, kp, 2, DL]

        oh = np.zeros((B_LOC, kpad, T), dtype=NP_FP8)
        bb, tt, cc = np.meshgrid(
            np.arange(B_LOC), np.arange(T), np.arange(C), indexing="ij"
        )
        oh[bb.ravel(), slots.ravel(), tt.ravel()] = fp8_one
        oh_up = np.ascontiguousarray(
            oh.reshape(B_LOC, kp, 2, 128, T).transpose(3, 0, 1, 2, 4)
        )  # [128, B_LOC, kp, 2, T]

        in_maps.append({"table": table_up, "onehot": oh_up, "eb": eb})
    return kp, in_maps


def kernel(x, level_hv, channel_hv, centroid):
    kp, in_maps = _host_prep(x, level_hv, channel_hv)
    if kp not in _CACHE:
        _CACHE[kp] = _build_program(kp)
    nc = _CACHE[kp]

    res = run_bass_kernel_spmd(nc, in_maps, list(range(N_CORES)))
    _CACHE["last_results"] = res
    _CACHE["nc"] = nc

    sample = np.empty((B, D), dtype=np.float32)
    for core in range(N_CORES):
        arr = res.results[core]["sample"]  # [80, 512]
        for b in range(B_LOC):
            row = arr[b * NCH : (b + 1) * NCH]  # [20, 512]
            for c in range(NCH):
                w = CHW_B[c]
                sample[core * B_LOC + b, CH * c : CH * c + w] = row[c, :w]
    sign = np.where(sample > 0, np.float32(1.0), np.float32(-1.0))
    return (sign @ np.asarray(centroid, dtype=np.float32).T).astype(np.float32)


# revision 38
# speedup vs baseline: 1.1212x; 1.0157x over previous
"""HDClassifier Trainium2 kernel (v2).

Math (per batch b):
  idx[t,c]   = clip(round((x+100)/200*200), 0, 200)
  bundled[t] = sum_c level_hv[idx[t,c]] * channel_hv[c]          # ints in [-8,8]
  gram[t',d] = prod_{i=0..3} bundled[t'+i, (d-(3-i)) mod D]      # |.| <= 4096
  sample[d]  = sum_{t'=0..124} gram[t',d]
  out        = sign(sample) @ centroid.T

Device strategy (8 cores, 4 batches each):
  - Host compacts the folded table per core: only the ~1350 (channel,level)
    keys actually used by the core's 4 batches are uploaded, padded to
    KP*256 rows (KP=6 expected) -> 6 DoubleRow k-passes instead of 7.
  - Rows carry a 3-col circular halo on the left (cols 9997..9999,0..9999),
    so the n-gram's circular d-shifts become plain free-dim offsets.
  - Phase A (chunk-major, table streamed): per 512-col chunk, per batch,
    KP fp8 DoubleRow matmuls accumulate onehot.T @ table in PSUM; Act
    drains to a per-batch fp8 bundled tile [128, 10003].
  - Phase B (per quarter, per batch): DMA-stage the partition-shifted
    bundled (sh1) and u (ush); DVE: u = bund*sh1 (fp16), gram = u*ush
    (fp16, values <= 4096 exact-ish in fp16).
  - t'-reduce: one fp16 matmul per (batch, chunk) with a ones-column
    lhsT view selecting row 20b+c of a single [80,512] f32 PSUM bank
    accumulated across all 80 matmuls -> one drain + one output DMA.
  - Host: sign + tiny [32,10000]@[10000,6] matmul.
"""

import sys

sys.path.insert(0, "/opt/trn_rl_repo")

import numpy as np

import concourse.bass as bass
import concourse.mybir as mybir
from concourse import bacc
from concourse.bass_utils import run_bass_kernel_spmd
from concourse.tile import TileContext

# Problem constants (hardcoded per contract)
NUM_LEVELS = 201
N_GRAM = 4
B, T, C, D, NUM_CLASSES = 32, 128, 8, 10000, 6
N_CORES = 8
B_LOC = B // N_CORES  # 4 batches per core
K_TOT = C * NUM_LEVELS  # 1608
HALO = N_GRAM - 1  # 3
DL = D + HALO  # 10003 local bundled width

CH = 512
NCH = 20  # out-chunk grid: 19x512 + 272 over D
CHW_B = [min(CH, D - CH * c) for c in range(NCH)]  # out-chunk widths (last 272)
# phase-A chunk grid over DL=10003: 19x512, a 3-col sliver, then 272.
# The sliver lets the 4th phase-B group (out-chunks 15-18, needing bundled
# cols <= 9730) start before the last 272-col chunk, so only the final
# 272-col group is exposed as tail.
A_STARTS = [CH * c for c in range(19)] + [9728, 9731]
A_WIDTHS = [CH] * 19 + [3, DL - 9731]
NCA = len(A_STARTS)  # 21
# phase-B slices: (first out-chunk, n chunks, ready after A-chunk index).
# A slice ending at out-chunk e reads bundled cols <= 512(e+1)+2, i.e.
# A-chunk e+1 (the sliver, index 19, for e=18; the last A-chunk for e=19).
# 3-chunk slices start phase B at A-chunk 3 and leave only a 272-col tail.
QDEF = [
    (0, 2, 2),
    (2, 2, 4),
    (4, 2, 6),
    (6, 2, 8),
    (8, 2, 10),
    (10, 2, 12),
    (12, 2, 14),
    (14, 2, 16),
    (16, 2, 18),
    (18, 1, 19),
    (19, 1, 20),
]


FP8 = mybir.dt.float8e4
FP16 = mybir.dt.float16
F32 = mybir.dt.float32
NP_FP8 = np.dtype(mybir.dt.np(FP8))
NP_FP16 = np.dtype(mybir.dt.np(FP16))

# tuning knobs (swept externally)
TAB_BUFS = 4
PSA_BUFS = 6
GRAM_BUFS = 3
STAGE_BUFS = 4
POOL_LS = (1,)  # which chunk indices of 3-chunk slices run their gram on Pool
FINE_FROM = 0  # out-chunks >= this are processed per-batch (endgame)
FINE_PAIR = True  # fine slices use batch-pairs instead of singles
FLUSH_AT = 0  # 0: after first pair's psh; 1: slice top; 2: after all pairs
POOL_PHASE = 1  # parity phase for pair/chunk Pool assignment
MERGE_GRAM = False  # one gram TT per pair covering the whole slice
DIRECT_OUT = False  # DMA psBIG (PSUM) straight to DRAM, skipping the drain

_CACHE = {}


def _build_program(kp):
    nc = bacc.Bacc("TRN2", target_bir_lowering=False, debug=False, num_devices=N_CORES)

    table_p = nc.declare_dram_parameter("table", [128, kp * 2, DL], FP8, isOutput=False)
    oh_p = nc.declare_dram_parameter("onehot", [128, B_LOC, kp, 2, T], FP8, isOutput=False)
    eb_p = nc.declare_dram_parameter("eb", [128, 2 * 80 + 1], FP16, isOutput=False)
    out_p = nc.declare_dram_parameter("sample", [80, CH], F32, isOutput=True)

    qmax = max(CH * (n - 1) + CHW_B[f + n - 1] for f, n, _ in QDEF) + 1
    with TileContext(nc) as tc:
        with (
            tc.tile_pool(name="const", bufs=1) as cpool,
            tc.tile_pool(name="tab", bufs=TAB_BUFS) as tpool,
            tc.tile_pool(name="bund", bufs=1) as bpool,
            tc.tile_pool(name="sh2", bufs=STAGE_BUFS) as shpool,
            tc.tile_pool(name="pp", bufs=STAGE_BUFS) as ppool,
            tc.tile_pool(name="psh", bufs=STAGE_BUFS) as pshpool,
            tc.tile_pool(name="gram", bufs=GRAM_BUFS) as gpool,
            tc.tile_pool(name="psA", bufs=PSA_BUFS, space="PSUM") as psA_pool,
            tc.tile_pool(name="psB", bufs=1, space="PSUM") as psB_pool,
        ):
            oh_sb = cpool.tile([128, B_LOC, kp, 2, T], FP8, tag="oh")
            nc.sync.dma_start(out=oh_sb[:, 0], in_=oh_p[:, 0])
            eb_sb = cpool.tile([128, 2 * 80 + 1], FP16, tag="eb")

            # all batches' bundled side by side: [t, b, col]
            bund = bpool.tile([128, B_LOC, DL], FP8, tag="bund", name="bund")
            psBIG = psB_pool.tile([80, CH], F32, tag="psBIG")
            nred = [0]  # count of reduce matmuls emitted

            deferred = []  # (gram tile, c, b, off) reduce matmuls, one slice late

            def emit_reduce(gram, c, b, off):
                w = CHW_B[c]
                r = b * NCH + c
                nc.tensor.matmul(
                    psBIG[:, 0:w],
                    eb_sb[:125, 80 - r : 160 - r],
                    gram[:125, b, off : off + w],
                    start=(nred[0] == 0),
                    stop=(nred[0] == B_LOC * NCH - 1),
                )
                nred[0] += 1

            def flush_deferred():
                while deferred:
                    emit_reduce(*deferred.pop(0))

            def phase_b_slice(q):
                """gram[t',d] = p[t',d] * p[t'+1,d+1] with
                p[t,m] = bund[t,m]*bund[t+2,m+2]; p is exact in fp8
                (products of even ints <= 64), so the staged shifts are fp8.
                ALL reduce matmuls are deferred one slice so the in-order PE
                queue never waits on a gram. Endgame slices (FINE_FROM) are
                processed per batch with alternating DVE/Pool grams to keep
                the dependency chains short where no phase-A work remains."""
                c_first, n_ch, _ = QDEF[q]
                q0 = CH * c_first  # d-offset of slice
                wq = CH * (n_ch - 1) + CHW_B[c_first + n_ch - 1]
                fine = c_first >= FINE_FROM
                # sh2[t, b, j] = bund[t+2, b, q0+j+2], j in [0, wq+1)
                sh2 = shpool.tile([128, B_LOC, qmax], FP8, tag="sh2")
                nc.sync.dma_start(
                    out=sh2[:126, :, 0 : wq + 1],
                    in_=bund[2:128, :, q0 + 2 : q0 + wq + 3],
                )
                p = ppool.tile([128, B_LOC, qmax], FP8, tag="p")
                psh = pshpool.tile([128, B_LOC, qmax - 1], FP8, tag="psh")
                if not fine:
                    bsplit = [range(B_LOC)]
                elif FINE_PAIR:
                    bsplit = [[0, 1], [2, 3]]
                else:
                    bsplit = [[b] for b in range(B_LOC)]
                if FLUSH_AT == 1:
                    flush_deferred()
                first = True
                for bs in bsplit:
                    blo, bhi = bs[0], bs[-1] + 1
                    # p[t, b, m] = bund[t, b, q0+m] * bund[t+2, b, q0+m+2]
                    nc.vector.tensor_mul(
                        out=p[:126, blo:bhi, 0 : wq + 1],
                        in0=bund[:126, blo:bhi, q0 : q0 + wq + 1],
                        in1=sh2[:126, blo:bhi, 0 : wq + 1],
                    )
                    # psh[t, b, j] = p[t+1, b, j+1]
                    nc.sync.dma_start(
                        out=psh[:125, blo:bhi, 0:wq],
                        in_=p[1:126, blo:bhi, 1 : wq + 1],
                    )
                    if first and FLUSH_AT == 0:
                        flush_deferred()
                    first = False
                    if MERGE_GRAM:
                        gram = gpool.tile([128, B_LOC, qmax - 1], FP16, tag="gram")
                        on_pool = fine and FINE_PAIR and (blo // 2) % 2 == POOL_PHASE
                        eng = nc.gpsimd if on_pool else nc.vector
                        eng.tensor_mul(
                            out=gram[:125, blo:bhi, 0:wq],
                            in0=p[:125, blo:bhi, 0:wq],
                            in1=psh[:125, blo:bhi, 0:wq],
                        )
                        for l in range(n_ch):
                            for b in bs:
                                deferred.append((gram, c_first + l, b, CH * l))
                        continue
                    for l in range(n_ch):
                        c = c_first + l
                        w = CHW_B[c]
                        off = CH * l
                        gram = gpool.tile([128, B_LOC, CH], FP16, tag="gram")
                        if fine:
                            on_pool = (
                                (blo // 2 + l) % 2 == POOL_PHASE
                                if FINE_PAIR
                                else blo % 2 == 1
                            )
                        else:
                            on_pool = n_ch == 3 and l in POOL_LS
                        eng = nc.gpsimd if on_pool else nc.vector
                        eng.tensor_mul(
                            out=gram[:125, blo:bhi, 0:w],
                            in0=p[:125, blo:bhi, off : off + w],
                            in1=psh[:125, blo:bhi, off : off + w],
                        )
                        for b in bs:
                            deferred.append((gram, c, b, 0))
                if FLUSH_AT == 2:
                    flush_deferred()

            schedule = {}
            for q, (_, _, ready) in enumerate(QDEF):
                schedule.setdefault(min(ready, NCA - 1), []).append(q)

            for ac in range(NCA):
                w = A_WIDTHS[ac]
                c0 = A_STARTS[ac]
                tab = tpool.tile([128, kp * 2, CH], FP8, tag="tab")
                nc.sync.dma_start(
                    out=tab[:, :, 0:w], in_=table_p[:, :, c0 : c0 + w]
                )
                if ac == 0:
                    for b in range(1, B_LOC):
                        nc.sync.dma_start(out=oh_sb[:, b], in_=oh_p[:, b])
                if ac == 1:
                    nc.sync.dma_start(out=eb_sb[:], in_=eb_p[:])
                for b in range(B_LOC):
                    ps = psA_pool.tile([128, w], F32, tag="psA", name=f"psA{ac}_{b}")
                    for k in range(kp):
                        nc.tensor.matmul(
                            ps[:],
                            oh_sb[:, b, k, :, :],
                            tab[:, 2 * k : 2 * k + 2, 0:w],
                            start=(k == 0),
                            stop=(k == kp - 1),
                            perf_mode=mybir.MatmulPerfMode.DoubleRow,
                        )
                    nc.scalar.copy(out=bund[:, b, c0 : c0 + w], in_=ps[:])
                for q in schedule.get(ac, []):
                    phase_b_slice(q)

            flush_deferred()
            if DIRECT_OUT:
                nc.sync.dma_start(out=out_p[:], in_=psBIG[:])
                samp = None
            else:
                samp = cpool.tile([80, CH], F32, tag="samp")
                nc.scalar.copy(out=samp[:], in_=psBIG[:])
            if samp is not None:
                nc.sync.dma_start(out=out_p[:], in_=samp[:])

    nc.finalize()
    return nc


def _host_prep(x, level_hv, channel_hv):
    # Bit-exact replication of the jax fp32 quantization
    x = np.asarray(x, dtype=np.float32)
    t1 = x + np.float32(100.0)
    t2 = t1 / np.float32(200.0)
    t3 = t2 * np.float32(200.0)
    idx = np.clip(np.rint(t3), 0, NUM_LEVELS - 1).astype(np.int32)  # [B,T,C]

    fp8_one = np.array([1.0], dtype=np.float32).astype(NP_FP8)[0]
    fp8_mone = np.array([-1.0], dtype=np.float32).astype(NP_FP8)[0]

    # folded +-1 table as fp8 bytes [1608, D]
    prod = (level_hv[None, :, :] * channel_hv[:, None, :]).reshape(K_TOT, D)
    F = np.where(prod > 0, fp8_one, fp8_mone)

    kk = np.arange(C, dtype=np.int32)[None, None, :] * NUM_LEVELS + idx  # [B,T,C]

    cores = []
    kp_max = 1
    for core in range(N_CORES):
        kk_c = kk[core * B_LOC : (core + 1) * B_LOC]  # [B_LOC, T, C]
        keys = np.unique(kk_c)
        n_k = len(keys)
        kp_c = -(-n_k // 256)
        kp_max = max(kp_max, kp_c)
        cores.append((kk_c, keys, n_k))

    kp = kp_max
    kpad = kp * 256
    in_maps = []
    eb = np.zeros((128, 2 * 80 + 1), dtype=NP_FP16)
    eb[: T - N_GRAM + 1, 80] = np.float16(1.0)
    for kk_c, keys, n_k in cores:
        inv = np.zeros(K_TOT, dtype=np.int32)
        inv[keys] = np.arange(n_k, dtype=np.int32)
        slots = inv[kk_c]  # [B_LOC, T, C]

        tabc = np.zeros((kpad, DL), dtype=NP_FP8)
        tabc[:n_k, HALO:] = F[keys]
        tabc[:n_k, :HALO] = F[keys][:, D - HALO :]
        table_up = np.ascontiguousarray(
            tabc.reshape(kp, 2, 128, DL).transpose(2, 0, 1, 3)
        )  # [128, kp, 2, DL]

        oh = np.zeros((B_LOC, kpad, T), dtype=NP_FP8)
        bb, tt, cc = np.meshgrid(
            np.arange(B_LOC), np.arange(T), np.arange(C), indexing="ij"
        )
        oh[bb.ravel(), slots.ravel(), tt.ravel()] = fp8_one
        oh_up = np.ascontiguousarray(
            oh.reshape(B_LOC, kp, 2, 128, T).transpose(3, 0, 1, 2, 4)
        )  # [128, B_LOC, kp, 2, T]

        in_maps.append({"table": table_up, "onehot": oh_up, "eb": eb})
    return kp, in_maps


def kernel(x, level_hv, channel_hv, centroid):
    kp, in_maps = _host_prep(x, level_hv, channel_hv)
    if kp not in _CACHE:
        _CACHE[kp] = _build_program(kp)
    nc = _CACHE[kp]

    res = run_bass_kernel_spmd(nc, in_maps, list(range(N_CORES)))
    _CACHE["last_results"] = res
    _CACHE["nc"] = nc

    sample = np.empty((B, D), dtype=np.float32)
    for core in range(N_CORES):
        arr = res.results[core]["sample"]  # [80, 512]
        for b in range(B_LOC):
            row = arr[b * NCH : (b + 1) * NCH]  # [20, 512]
            for c in range(NCH):
                w = CHW_B[c]
                sample[core * B_LOC + b, CH * c : CH * c + w] = row[c, :w]
    sign = np.where(sample > 0, np.float32(1.0), np.float32(-1.0))
    return (sign @ np.asarray(centroid, dtype=np.float32).T).astype(np.float32)
